# revision 3
# baseline (speedup 1.0000x reference)
"""InfoNCE loss kernel for 8 Trainium2 NeuronCores.

Math (reference): z = concat(z1, z2) [2N, D] row-normalized; sim = z@z.T/TEMP;
self-diagonal masked; loss = mean(-pos + logsumexp(sim, axis=1)) where
pos[i] = sim[i, partner(i)].

Sharding: data-parallel over the 2N row dimension - core c owns rows
[c*1024, (c+1)*1024). Each core computes its [1024, 8192] block of sim as
1024 (K=128, M=128, N=512) bf16 matmuls against the full z, fuses
exp(x/TEMP - 1/TEMP) + row-sum on the scalar engine, extracts the positive
diagonal and masks the self diagonal on the vector engine, and returns
per-row (ln(S_r) - pos_r). Host adds the constant M = 1/TEMP shift and
takes the mean.

Trick: columns of z are permuted per-core so the self block is always
block 0 and the positive-partner block is always block 1 - making the
SPMD graph identical across cores (diag offsets become compile-time
constants).
"""

from contextlib import ExitStack

import ml_dtypes
import numpy as np

import concourse.bass as bass
import concourse.tile as tile
from concourse import bacc, mybir
from concourse.bass_utils import run_bass_kernel_spmd
from concourse.masks import make_identity

N_CORES = 8
N, D = 4096, 1024
ROWS = 2 * N               # 8192 total rows of z
RPC = ROWS // N_CORES      # 1024 rows per core
TEMP = 0.07
INV_T = 1.0 / TEMP
NTILE = 512                # matmul free dim / PSUM bank (fp32)
NT = ROWS // NTILE         # 16 column tiles
MT = RPC // 128            # 8 row tiles per core
KT = D // 128              # 8 contraction tiles

_CACHE = {}


def _build_graph():
    nc = bacc.Bacc("TRN2", target_bir_lowering=False, debug=False, num_devices=N_CORES)
    z = nc.declare_dram_parameter("z", [D, ROWS], mybir.dt.bfloat16, isOutput=False)
    out = nc.declare_dram_parameter("out", [128, MT], mybir.dt.float32, isOutput=True)

    fp32 = mybir.dt.float32
    AF = mybir.ActivationFunctionType
    AX = mybir.AxisListType.X

    with tile.TileContext(nc) as tc, ExitStack() as ctx:
        zpool = ctx.enter_context(tc.tile_pool(name="z", bufs=1))
        consts = ctx.enter_context(tc.tile_pool(name="consts", bufs=1))
        pspool = ctx.enter_context(tc.tile_pool(name="ps", bufs=2, space="PSUM"))
        expool = ctx.enter_context(tc.tile_pool(name="ex", bufs=6))
        accpool = ctx.enter_context(tc.tile_pool(name="acc", bufs=2))
        redpool = ctx.enter_context(tc.tile_pool(name="red", bufs=4))
        outpool = ctx.enter_context(tc.tile_pool(name="outp", bufs=1))

        # constants: identity (positive extraction), 1-identity (self mask),
        # bias column of -1/TEMP for the exp shift
        eye = consts.tile([128, 128], fp32, tag="eye")
        make_identity(nc, eye[:])
        aeye = consts.tile([128, 128], fp32, tag="aeye")
        nc.gpsimd.memset(aeye[:], 1.0)
        nc.gpsimd.affine_select(
            out=aeye[:],
            in_=aeye[:],
            compare_op=mybir.AluOpType.not_equal,
            fill=0.0,
            base=0,
            pattern=[[-1, 128]],
            channel_multiplier=1,
        )
        nbias = consts.tile([128, 1], fp32, tag="nbias")
        nc.vector.memset(nbias[:], -INV_T)

        outsb = outpool.tile([128, MT], fp32, tag="outsb")

        # stage all of z into SBUF as [128, 512] bf16 tiles (16 MB total)
        zt = [[None] * NT for _ in range(KT)]
        for c in range(NT):
            for k in range(KT):
                t = zpool.tile([128, NTILE], mybir.dt.bfloat16, tag=f"z{k}_{c}", name=f"z{k}_{c}")
                nc.sync.dma_start(
                    out=t[:], in_=z[k * 128 : (k + 1) * 128, c * NTILE : (c + 1) * NTILE]
                )
                zt[k][c] = t

        for mt in range(MT):
            cself = mt // 4              # column tile holding this m-tile's self diag
            off = (mt % 4) * 128         # diag offset within that 512-wide tile
            nt_diag = cself
            nt_par = (NT // 8) * 2 + cself - cself  # placeholder, fixed below
            nt_par = 2 + cself           # partner block = cols [1024, 2048)

            acc = accpool.tile([128, NT], fp32, tag="acc")
            dotcol = redpool.tile([128, 1], fp32, tag="dot")

            for ng in range(NT // 4):
                ps = [pspool.tile([128, NTILE], fp32, tag=f"ps{j}", name=f"ps{j}") for j in range(4)]
                for k in range(KT):
                    lt = zt[k][cself][:, off : off + 128]
                    for j in range(4):
                        nt = ng * 4 + j
                        nc.tensor.matmul(
                            ps[j][:],
                            lhsT=lt,
                            rhs=zt[k][nt][:],
                            start=(k == 0),
                            stop=(k == KT - 1),
                        )
                for j in range(4):
                    nt = ng * 4 + j
                    ex = expool.tile([128, NTILE], fp32, tag="ex")
                    if nt == nt_diag:
                        # self tile: exp, zero the self-diagonal, then row-sum
                        nc.scalar.activation(
                            out=ex[:], in_=ps[j][:], func=AF.Exp,
                            bias=nbias[:], scale=INV_T,
                        )
                        nc.vector.tensor_mul(
                            ex[:, off : off + 128], ex[:, off : off + 128], aeye[:]
                        )
                        nc.vector.reduce_sum(acc[:, nt : nt + 1], ex[:], axis=AX)
                    elif nt == nt_par:
                        # partner tile: extract positive diag from raw psum,
                        # then fused exp + row-sum (positives stay in denominator)
                        pm = expool.tile([128, 128], fp32, tag="pm")
                        nc.vector.tensor_mul(pm[:], ps[j][:, off : off + 128], eye[:])
                        nc.vector.reduce_sum(dotcol[:], pm[:], axis=AX)
                        nc.scalar.activation(
                            out=ex[:], in_=ps[j][:], func=AF.Exp,
                            bias=nbias[:], scale=INV_T,
                            accum_out=acc[:, nt : nt + 1],
                        )
                    else:
                        nc.scalar.activation(
                            out=ex[:], in_=ps[j][:], func=AF.Exp,
                            bias=nbias[:], scale=INV_T,
                            accum_out=acc[:, nt : nt + 1],
                        )

            S = redpool.tile([128, 1], fp32, tag="S")
            nc.vector.reduce_sum(S[:], acc[:], axis=AX)
            lnS = redpool.tile([128, 1], fp32, tag="lnS")
            nc.scalar.activation(out=lnS[:], in_=S[:], func=AF.Ln, bias=0.0, scale=1.0)
            # out[:, mt] = ln(S) - INV_T * dot_partner
            nc.scalar.activation(
                out=outsb[:, mt : mt + 1], in_=dotcol[:], func=AF.Identity,
                bias=lnS[:], scale=-INV_T,
            )

        nc.sync.dma_start(out=out[:], in_=outsb[:])

    nc.compile()
    return nc


def kernel(z1: np.ndarray, z2: np.ndarray) -> np.ndarray:
    assert z1.shape == (N, D) and z2.shape == (N, D)
    z = np.concatenate([z1, z2], axis=0)  # [8192, 1024] f32

    # per-core column permutation: [self block, partner block, rest]
    in_maps = []
    for c in range(N_CORES):
        p = (c + 4) % N_CORES
        order = [c, p] + [b for b in range(N_CORES) if b != c and b != p]
        idx = np.concatenate([np.arange(b * RPC, (b + 1) * RPC) for b in order])
        zc = np.ascontiguousarray(z[idx].T.astype(ml_dtypes.bfloat16))  # [1024, 8192]
        in_maps.append({"z": zc})

    if "nc" not in _CACHE:
        _CACHE["nc"] = _build_graph()
    res = run_bass_kernel_spmd(_CACHE["nc"], in_maps, core_ids=list(range(N_CORES)))

    total = 0.0
    for r in res.results:
        total += float(np.asarray(r["out"], dtype=np.float64).sum())
    return np.asarray(INV_T + total / ROWS, dtype=np.float32)


# revision 5
# speedup vs baseline: 2.0075x; 2.0075x over previous
"""InfoNCE loss kernel for 8 Trainium2 NeuronCores (fp8 DoubleRow version).

Math (reference): z = concat(z1, z2) [2N, D] row-normalized; sim = z@z.T/TEMP;
self-diagonal masked; loss = mean(-pos + logsumexp(sim, axis=1)) where
pos[i] = sim[i, partner(i)].

Sharding: data-parallel over the 2N row dimension - core c owns rows
[c*1024, (c+1)*1024). Each core computes its [1024, 8192] block of sim
against the full z with fp8e4m3 DoubleRow matmuls (256-deep contraction per
instruction -> 512 matmuls/core), applies exp(x/TEMP - 1/TEMP) on the scalar
engine over two PSUM banks at a time, row-reduces on the vector engine,
extracts the positive diagonal from raw PSUM, and returns per-row
(ln(S_r) - pos_r). Host adds the constant 1/TEMP shift and takes the mean.

Tricks:
- columns of z are permuted per-core so the self block is always block 0 and
  the positive-partner block is always block 1, making the SPMD graph
  identical across cores (diag offsets are compile-time constants).
- z is pre-scaled by 8 on the host before the fp8 cast (uses the e4m3
  dynamic range); the 1/64 is folded into the exp scale.
"""

from contextlib import ExitStack

import ml_dtypes
import numpy as np

import concourse.bass as bass
import concourse.tile as tile
from concourse import bacc, mybir
from concourse.bass_utils import run_bass_kernel_spmd
from concourse.masks import make_identity

N_CORES = 8
N, D = 4096, 1024
ROWS = 2 * N               # 8192 total rows of z
RPC = ROWS // N_CORES      # 1024 rows per core
TEMP = 0.07
INV_T = 1.0 / TEMP
FP8_SCALE = 8.0            # host pre-scale before e4m3 cast
MM_SCALE = INV_T / (FP8_SCALE * FP8_SCALE)
NTILE = 512                # columns per z SBUF tile / PSUM bank (fp32)
NT = ROWS // NTILE         # 16 column tiles
MT = RPC // 128            # 8 row tiles per core
KT = D // 128              # 8 contraction slices (4 DoubleRow pairs)
NPAIR = NT // 2            # 8 column-tile pairs per row tile

_CACHE = {}


def _build_graph():
    nc = bacc.Bacc("TRN2", target_bir_lowering=False, debug=False, num_devices=N_CORES)
    z = nc.declare_dram_parameter("z", [D, ROWS], mybir.dt.float8e4, isOutput=False)
    out = nc.declare_dram_parameter("out", [128, MT], mybir.dt.float32, isOutput=True)

    fp32 = mybir.dt.float32
    bf16 = mybir.dt.bfloat16
    fp8 = mybir.dt.float8e4
    AF = mybir.ActivationFunctionType
    AX = mybir.AxisListType.X
    DR = mybir.MatmulPerfMode.DoubleRow

    with tile.TileContext(nc) as tc, ExitStack() as ctx:
        zpool = ctx.enter_context(tc.tile_pool(name="z", bufs=1))
        consts = ctx.enter_context(tc.tile_pool(name="consts", bufs=1))
        pspool = ctx.enter_context(tc.tile_pool(name="ps", bufs=4, space="PSUM"))
        expool = ctx.enter_context(tc.tile_pool(name="ex", bufs=4))
        pmpool = ctx.enter_context(tc.tile_pool(name="pm", bufs=4))
        accpool = ctx.enter_context(tc.tile_pool(name="acc", bufs=2))
        outpool = ctx.enter_context(tc.tile_pool(name="outp", bufs=1))

        # constants: identity (positive extraction), 1-identity (self mask,
        # bf16 to match the exp tiles), bias column of -1/TEMP
        eye = consts.tile([128, 128], fp32, tag="eye")
        make_identity(nc, eye[:])
        aeye = consts.tile([128, 128], bf16, tag="aeye")
        nc.gpsimd.memset(aeye[:], 1.0)
        nc.gpsimd.affine_select(
            out=aeye[:],
            in_=aeye[:],
            compare_op=mybir.AluOpType.not_equal,
            fill=0.0,
            base=0,
            pattern=[[-1, 128]],
            channel_multiplier=1,
        )
        nbias = consts.tile([128, 1], fp32, tag="nbias")
        nc.vector.memset(nbias[:], -INV_T)

        Scol = outpool.tile([128, MT], fp32, tag="Scol")      # per-row exp sums
        dotcols = outpool.tile([128, MT], fp32, tag="dotcols")  # raw positive dots
        outsb = outpool.tile([128, MT], fp32, tag="outsb")

        # stage z into SBUF: one [128, KT, 512] fp8 tile per column block,
        # one 512KB DMA each (batched to keep sync-engine issue cost low)
        zsrc = z.rearrange("(k p) n -> p k n", p=128)
        zc = []
        for c in range(NT):
            t = zpool.tile([128, KT, NTILE], fp8, tag=f"zc{c}", name=f"zc{c}")
            nc.sync.dma_start(out=t[:], in_=zsrc[:, :, c * NTILE : (c + 1) * NTILE])
            zc.append(t)

        for mt in range(MT):
            cself = mt // 4              # column tile holding this m-tile's diag
            off = (mt % 4) * 128         # diag offset within that 512-wide tile
            # pair 0 (cols 0..1023) holds the self diag; pair 1 (cols
            # 1024..2047) holds the positive-partner diag
            dofh = cself                 # which half of the pair tile
            acc = accpool.tile([128, NT], fp32, tag="acc")

            for p in range(NPAIR):
                ps = pspool.tile([128, 2, NTILE], fp32, tag="ps", name="ps")
                for k in range(KT // 2):
                    lt = zc[cself][:, 2 * k : 2 * k + 2, off : off + 128]
                    for h in range(2):
                        nc.tensor.matmul(
                            ps[:, h, :],
                            lhsT=lt,
                            rhs=zc[2 * p + h][:, 2 * k : 2 * k + 2, :],
                            start=(k == 0),
                            stop=(k == KT // 2 - 1),
                            perf_mode=DR,
                        )
                if p == 1:
                    # positive-partner diag: extract raw dot from PSUM
                    pm = pmpool.tile([128, 128], fp32, tag="pm")
                    nc.vector.tensor_mul(pm[:], ps[:, dofh, off : off + 128], eye[:])
                    nc.vector.reduce_sum(dotcols[:, mt : mt + 1], pm[:], axis=AX)
                ex = expool.tile([128, 2, NTILE], bf16, tag="ex")
                nc.scalar.activation(
                    out=ex[:], in_=ps[:], func=AF.Exp, bias=nbias[:], scale=MM_SCALE
                )
                if p == 0:
                    # zero the self-diagonal before the row-sum
                    nc.vector.tensor_mul(
                        ex[:, dofh, off : off + 128],
                        ex[:, dofh, off : off + 128],
                        aeye[:],
                    )
                nc.vector.reduce_sum(acc[:, 2 * p : 2 * p + 2], ex[:], axis=AX)

            nc.vector.reduce_sum(Scol[:, mt : mt + 1], acc[:], axis=AX)

        # tail: ln(S) and combine (kept out of the loop so the scalar engine
        # doesn't thrash activation tables between Exp and Ln)
        lnS = outpool.tile([128, MT], fp32, tag="lnS")
        nc.scalar.activation(out=lnS[:], in_=Scol[:], func=AF.Ln, bias=0.0, scale=1.0)
        dsc = outpool.tile([128, MT], fp32, tag="dsc")
        nc.scalar.activation(
            out=dsc[:], in_=dotcols[:], func=AF.Identity, bias=0.0, scale=-MM_SCALE
        )
        nc.vector.tensor_add(outsb[:], lnS[:], dsc[:])
        nc.sync.dma_start(out=out[:], in_=outsb[:])

    nc.compile()
    return nc


def _make_in_maps(z1: np.ndarray, z2: np.ndarray):
    z = np.concatenate([z1, z2], axis=0)  # [8192, 1024] f32
    # per-core column permutation: [self block, partner block, rest]
    in_maps = []
    zs = (z * FP8_SCALE).astype(np.float32)
    for c in range(N_CORES):
        p = (c + 4) % N_CORES
        order = [c, p] + [b for b in range(N_CORES) if b != c and b != p]
        idx = np.concatenate([np.arange(b * RPC, (b + 1) * RPC) for b in order])
        zcb = np.ascontiguousarray(zs[idx].T.astype(ml_dtypes.float8_e4m3))
        in_maps.append({"z": zcb})
    return in_maps


def kernel(z1: np.ndarray, z2: np.ndarray) -> np.ndarray:
    assert z1.shape == (N, D) and z2.shape == (N, D)
    in_maps = _make_in_maps(z1, z2)

    if "nc" not in _CACHE:
        _CACHE["nc"] = _build_graph()
    res = run_bass_kernel_spmd(_CACHE["nc"], in_maps, core_ids=list(range(N_CORES)))

    total = 0.0
    for r in res.results:
        total += float(np.asarray(r["out"], dtype=np.float64).sum())
    return np.asarray(INV_T + total / ROWS, dtype=np.float32)


# revision 6
# speedup vs baseline: 2.1127x; 1.0524x over previous
"""InfoNCE loss kernel for 8 Trainium2 NeuronCores (fp8 DoubleRow version).

Math (reference): z = concat(z1, z2) [2N, D] row-normalized; sim = z@z.T/TEMP;
self-diagonal masked; loss = mean(-pos + logsumexp(sim, axis=1)) where
pos[i] = sim[i, partner(i)].

Sharding: data-parallel over the 2N row dimension - core c owns rows
[c*1024, (c+1)*1024). Each core computes its [1024, 8192] block of sim
against the full z with fp8e4m3 DoubleRow matmuls (256-deep contraction per
instruction -> 512 matmuls/core), applies exp(x/TEMP - 1/TEMP) on the scalar
engine over two PSUM banks at a time, row-reduces on the vector engine,
extracts the positive diagonal from raw PSUM, and returns per-row
(ln(S_r) - pos_r). Host adds the constant 1/TEMP shift and takes the mean.

Tricks:
- columns of z are permuted per-core so the self block is always block 0 and
  the positive-partner block is always block 1, making the SPMD graph
  identical across cores (diag offsets are compile-time constants).
- z is pre-scaled by 8 on the host before the fp8 cast (uses the e4m3
  dynamic range); the 1/64 is folded into the exp scale.
"""

from contextlib import ExitStack

import ml_dtypes
import numpy as np

import concourse.bass as bass
import concourse.tile as tile
from concourse import bacc, mybir
from concourse.bass_utils import run_bass_kernel_spmd
from concourse.masks import make_identity

N_CORES = 8
N, D = 4096, 1024
ROWS = 2 * N               # 8192 total rows of z
RPC = ROWS // N_CORES      # 1024 rows per core
TEMP = 0.07
INV_T = 1.0 / TEMP
FP8_SCALE = 8.0            # host pre-scale before e4m3 cast
MM_SCALE = INV_T / (FP8_SCALE * FP8_SCALE)
NTILE = 512                # columns per z SBUF tile / PSUM bank (fp32)
NT = ROWS // NTILE         # 16 column tiles
MT = RPC // 128            # 8 row tiles per core
KT = D // 128              # 8 contraction slices (4 DoubleRow pairs)
NPAIR = NT // 2            # 8 column-tile pairs per row tile

_CACHE = {}


def _build_graph():
    nc = bacc.Bacc("TRN2", target_bir_lowering=False, debug=False, num_devices=N_CORES)
    z = nc.declare_dram_parameter("z", [NT, 128, KT, NTILE], mybir.dt.float8e4, isOutput=False)
    out = nc.declare_dram_parameter("out", [128, MT], mybir.dt.float32, isOutput=True)

    fp32 = mybir.dt.float32
    bf16 = mybir.dt.bfloat16
    fp8 = mybir.dt.float8e4
    AF = mybir.ActivationFunctionType
    AX = mybir.AxisListType.X
    DR = mybir.MatmulPerfMode.DoubleRow

    with tile.TileContext(nc) as tc, ExitStack() as ctx:
        zpool = ctx.enter_context(tc.tile_pool(name="z", bufs=1))
        consts = ctx.enter_context(tc.tile_pool(name="consts", bufs=1))
        pspool = ctx.enter_context(tc.tile_pool(name="ps", bufs=4, space="PSUM"))
        expool = ctx.enter_context(tc.tile_pool(name="ex", bufs=4))
        pmpool = ctx.enter_context(tc.tile_pool(name="pm", bufs=4))
        accpool = ctx.enter_context(tc.tile_pool(name="acc", bufs=2))
        outpool = ctx.enter_context(tc.tile_pool(name="outp", bufs=1))

        # constants: identity (positive extraction), 1-identity (self mask,
        # bf16 to match the exp tiles), bias column of -1/TEMP
        eye = consts.tile([128, 128], fp32, tag="eye")
        make_identity(nc, eye[:])
        aeye = consts.tile([128, 128], bf16, tag="aeye")
        nc.gpsimd.memset(aeye[:], 1.0)
        nc.gpsimd.affine_select(
            out=aeye[:],
            in_=aeye[:],
            compare_op=mybir.AluOpType.not_equal,
            fill=0.0,
            base=0,
            pattern=[[-1, 128]],
            channel_multiplier=1,
        )
        nbias = consts.tile([128, 1], fp32, tag="nbias")
        nc.vector.memset(nbias[:], -INV_T)

        Scol = outpool.tile([128, MT], fp32, tag="Scol")      # per-row exp sums
        dotcols = outpool.tile([128, MT], fp32, tag="dotcols")  # raw positive dots
        outsb = outpool.tile([128, MT], fp32, tag="outsb")

        # stage z into SBUF: one [128, KT, 512] fp8 tile per column block,
        # one 512KB DMA each (batched to keep sync-engine issue cost low)
        zc = []
        for c in range(NT):
            t = zpool.tile([128, KT, NTILE], fp8, tag=f"zc{c}", name=f"zc{c}")
            nc.sync.dma_start(out=t[:], in_=z[c])
            zc.append(t)

        for mt in range(MT):
            cself = mt // 4              # column tile holding this m-tile's diag
            off = (mt % 4) * 128         # diag offset within that 512-wide tile
            # pair 0 (cols 0..1023) holds the self diag; pair 1 (cols
            # 1024..2047) holds the positive-partner diag
            dofh = cself                 # which half of the pair tile
            acc = accpool.tile([128, NPAIR], fp32, tag="acc")

            for p in range(NPAIR):
                ps = pspool.tile([128, 2, NTILE], fp32, tag="ps", name="ps")
                for k in range(KT // 2):
                    lt = zc[cself][:, 2 * k : 2 * k + 2, off : off + 128]
                    for h in range(2):
                        nc.tensor.matmul(
                            ps[:, h, :],
                            lhsT=lt,
                            rhs=zc[2 * p + h][:, 2 * k : 2 * k + 2, :],
                            start=(k == 0),
                            stop=(k == KT // 2 - 1),
                            perf_mode=DR,
                        )
                if p == 1:
                    # positive-partner diag: extract raw dot from PSUM
                    pm = pmpool.tile([128, 128], fp32, tag="pm")
                    nc.vector.tensor_mul(pm[:], ps[:, dofh, off : off + 128], eye[:])
                    nc.vector.reduce_sum(dotcols[:, mt : mt + 1], pm[:], axis=AX)
                ex = expool.tile([128, 2, NTILE], bf16, tag="ex")
                if p == 0:
                    # self pair: exp, zero the self-diagonal, then row-sum on DVE
                    nc.scalar.activation(
                        out=ex[:], in_=ps[:], func=AF.Exp, bias=nbias[:], scale=MM_SCALE
                    )
                    nc.vector.tensor_mul(
                        ex[:, dofh, off : off + 128],
                        ex[:, dofh, off : off + 128],
                        aeye[:],
                    )
                    nc.vector.reduce_sum(
                        acc[:, 0:1], ex[:], axis=mybir.AxisListType.XY
                    )
                else:
                    # fused exp + row-sum on the scalar engine
                    nc.scalar.activation(
                        out=ex[:], in_=ps[:], func=AF.Exp, bias=nbias[:],
                        scale=MM_SCALE, accum_out=acc[:, p : p + 1],
                    )

            nc.vector.reduce_sum(Scol[:, mt : mt + 1], acc[:], axis=AX)

        # tail: ln(S) and combine (kept out of the loop so the scalar engine
        # doesn't thrash activation tables between Exp and Ln)
        lnS = outpool.tile([128, MT], fp32, tag="lnS")
        nc.scalar.activation(out=lnS[:], in_=Scol[:], func=AF.Ln, bias=0.0, scale=1.0)
        dsc = outpool.tile([128, MT], fp32, tag="dsc")
        nc.scalar.activation(
            out=dsc[:], in_=dotcols[:], func=AF.Identity, bias=0.0, scale=-MM_SCALE
        )
        nc.vector.tensor_add(outsb[:], lnS[:], dsc[:])
        nc.sync.dma_start(out=out[:], in_=outsb[:])

    nc.compile()
    return nc


def _make_in_maps(z1: np.ndarray, z2: np.ndarray):
    z = np.concatenate([z1, z2], axis=0)  # [8192, 1024] f32
    # per-core column permutation: [self block, partner block, rest]
    in_maps = []
    zs = (z * FP8_SCALE).astype(np.float32)
    for c in range(N_CORES):
        p = (c + 4) % N_CORES
        order = [c, p] + [b for b in range(N_CORES) if b != c and b != p]
        idx = np.concatenate([np.arange(b * RPC, (b + 1) * RPC) for b in order])
        zt = zs[idx].T  # [D, ROWS] permuted
        # [NT, 128, KT, NTILE]: per column-block, contiguous [p, k, n] tiles
        zcb = np.ascontiguousarray(
            zt.reshape(KT, 128, NT, NTILE).transpose(2, 1, 0, 3)
        ).astype(ml_dtypes.float8_e4m3)
        in_maps.append({"z": zcb})
    return in_maps


def kernel(z1: np.ndarray, z2: np.ndarray) -> np.ndarray:
    assert z1.shape == (N, D) and z2.shape == (N, D)
    in_maps = _make_in_maps(z1, z2)

    if "nc" not in _CACHE:
        _CACHE["nc"] = _build_graph()
    res = run_bass_kernel_spmd(_CACHE["nc"], in_maps, core_ids=list(range(N_CORES)))

    total = 0.0
    for r in res.results:
        total += float(np.asarray(r["out"], dtype=np.float64).sum())
    return np.asarray(INV_T + total / ROWS, dtype=np.float32)
